# revision 16
# baseline (speedup 1.0000x reference)
"""Trainium2 Bass kernel for nn_CrossTransformerBlock (sparse kNN cross-attention).

Shapes (hardcoded): B=4, NQ=4096, N=2048, DIM=128, DG=256, DI=256, K=16.
Sharding: 8 cores = (batch b, query-half h); each core handles 2048 queries
against its batch's 2048 points.

Per-core design (software-pipelined over 8 blocks of 256 queries):
  prep:  PE projects k_all/v_all (points @ W), g_all = W_d1@xyz into a
         row-major DRAM table [2048, 384] f16 (k|v|g); u deferred.
  topk:  PE computes s = 2 q.x - |x|^2 per 512-col chunk; DVE ORs the
         GLOBAL 11-bit column index into the mantissa low bits, then
         per-128-chunk top-8 (max8) + match_replace merge -> top-16 packed;
         idx = value & 0x7FF (one STT; no max_index needed).
  gather: selector matmul -> wrapped i16 layout; dma_gather(transpose=True)
         pulls 16 rows/query feature-major; 32.4us/block on GpSimd = the
         pipeline bottleneck; issue order keeps gathers back-to-back.
  mlp:   h=relu(u-g); pos=W_d2 h+b; g1=relu(W_g1(pos-k)+b_g1') with
         q_attn folded into b_g1' on host; exp on ACT evac; softmax sums
         via f16 tree adds (eg folded via STT); global slot via eg/egv.
  Loop issue order per iteration: topk(it) -> sel+gather(it) -> mlp(it-1),
  so the gather-gating chain never sits behind gather-dependent MLP ops
  in the in-order per-engine queues.
"""

import numpy as np

import concourse.bass as bass
import concourse.bacc as bacc
import concourse.mybir as mybir
from concourse.tile import TileContext
from concourse.bass_utils import run_bass_kernel_spmd

F32 = mybir.dt.float32
F16 = mybir.dt.float16
U32 = mybir.dt.uint32
I32 = mybir.dt.int32
I16 = mybir.dt.int16
ALU = mybir.AluOpType
ACTF = mybir.ActivationFunctionType

B, NQ, N, DIM, DG, DI, K = 4, 4096, 2048, 128, 256, 256, 16
NQC = 2048          # queries per core
QTILE = 128         # topk tile (queries on partitions)
QBLK = 256          # gather/MLP block
NBLK = NQC // QBLK
CHUNK = 256         # topk candidate chunk (top-8 per chunk)
NCHUNK = N // CHUNK
ROWF = 3 * DIM      # table row features (k|v|g)

_CACHE = {}


def _build():
    nc = bacc.Bacc("TRN2", target_bir_lowering=False, debug=False, num_devices=8)

    # ---- external inputs (per core) ----
    qx4 = nc.dram_tensor("qx4", [4, NQC], F32, kind="ExternalInput")
    xt4 = nc.dram_tensor("xt4", [4, N], F32, kind="ExternalInput")
    ptsT = nc.dram_tensor("ptsT", [128, 2 * N], F16, kind="ExternalInput")
    xyzq4 = nc.dram_tensor("xyzq4", [4, NQC], F16, kind="ExternalInput")
    xyzn4 = nc.dram_tensor("xyzn4", [4, N], F16, kind="ExternalInput")
    wk_l = nc.dram_tensor("wk_l", [128, 2 * DIM], F16, kind="ExternalInput")
    wv_l = nc.dram_tensor("wv_l", [128, 2 * DIM], F16, kind="ExternalInput")
    wd1_l = nc.dram_tensor("wd1_l", [4, DIM], F16, kind="ExternalInput")
    wd2_l = nc.dram_tensor("wd2_l", [DIM, DIM], F16, kind="ExternalInput")
    wg1_l = nc.dram_tensor("wg1_l", [DIM, DIM], F16, kind="ExternalInput")
    wg2_l = nc.dram_tensor("wg2_l", [DIM, DIM], F16, kind="ExternalInput")
    # per-partition column vectors [128, 5] f32: b_d2, b_g1', b_g2, eg, egv
    colv = nc.dram_tensor("colv", [DIM, 5], F32, kind="ExternalInput")
    # constants
    esel = nc.dram_tensor("esel", [128, 128], F16, kind="ExternalInput")
    masks = nc.dram_tensor("masks", [128, 2 * 256], F16, kind="ExternalInput")
    ident16 = nc.dram_tensor("ident16", [128, 128], F16, kind="ExternalInput")
    ident32 = nc.dram_tensor("ident32", [128, 128], F32, kind="ExternalInput")
    iota_d = nc.dram_tensor("iota_d", [128, 512], I32, kind="ExternalInput")

    out = nc.dram_tensor("out", [NQC, DIM], F32, kind="ExternalOutput")

    with TileContext(nc) as tc:
        with tc.tile_pool(name="const", bufs=1) as cpool, \
             tc.tile_pool(name="prep", bufs=1) as prep, \
             tc.tile_pool(name="work", bufs=2) as work, \
             tc.tile_pool(name="mlp", bufs=1) as mlp, \
             tc.tile_pool(name="psk", bufs=2, space="PSUM") as psk, \
             tc.tile_pool(name="psm", bufs=3, space="PSUM") as psm, \
             tc.tile_pool(name="pss", bufs=2, space="PSUM") as pss, \
             tc.tile_pool(name="dram", bufs=1, space="DRAM") as dpool:

            # ---------- load operands (order = need order) ----------
            qx4_s = cpool.tile([4, NQC], F32)
            xt4_s = cpool.tile([4, N], F32)
            nc.sync.dma_start(qx4_s[:], qx4[:])
            nc.sync.dma_start(xt4_s[:], xt4[:])
            pts_s = cpool.tile([128, 2 * N], F16)
            nc.sync.dma_start(pts_s[:], ptsT[:])
            xyzn4_s = cpool.tile([4, N], F16)
            nc.sync.dma_start(xyzn4_s[:], xyzn4[:])
            wk_s = cpool.tile([128, 2 * DIM], F16)
            wv_s = cpool.tile([128, 2 * DIM], F16)
            nc.sync.dma_start(wk_s[:], wk_l[:])
            nc.sync.dma_start(wv_s[:], wv_l[:])
            wd1_s = cpool.tile([4, DIM], F16)
            nc.sync.dma_start(wd1_s[:], wd1_l[:])
            id16 = cpool.tile([128, 128], F16)
            nc.sync.dma_start(id16[:], ident16[:])
            iota = cpool.tile([128, 512], I32)
            nc.sync.dma_start(iota[:], iota_d[:])
            esel_s = cpool.tile([128, 128], F16)
            nc.sync.dma_start(esel_s[:], esel[:])
            masks_s = cpool.tile([128, 2 * 256], F16)
            nc.sync.dma_start(masks_s[:], masks[:])
            xyzq4_s = cpool.tile([4, NQC], F16)
            nc.sync.dma_start(xyzq4_s[:], xyzq4[:])
            wd2_s = cpool.tile([DIM, DIM], F16)
            wg1_s = cpool.tile([DIM, DIM], F16)
            wg2_s = cpool.tile([DIM, DIM], F16)
            nc.sync.dma_start(wd2_s[:], wd2_l[:])
            nc.sync.dma_start(wg1_s[:], wg1_l[:])
            nc.sync.dma_start(wg2_s[:], wg2_l[:])
            colv_s = cpool.tile([DIM, 5], F32)
            nc.sync.dma_start(colv_s[:], colv[:])
            b_d2 = colv_s[:, 0:1]
            b_g1 = colv_s[:, 1:2]
            b_g2 = colv_s[:, 2:3]
            eg = colv_s[:, 3:4]
            egv = colv_s[:, 4:5]
            id32 = cpool.tile([128, 128], F32)
            nc.sync.dma_start(id32[:], ident32[:])

            # pack mask / idx mask columns + shift counts for STT forms
            bitc = cpool.tile([128, 3], U32)
            nc.vector.memset(bitc[:, 0:1], 0xFFFFFF00)
            nc.vector.memset(bitc[:, 1:2], 0xFF)
            nc.vector.memset(bitc[:, 2:3], 0x78)
            sh16 = cpool.tile([128, 16], U32)
            nc.vector.memset(sh16[:], 5)
            bitd = cpool.tile([128, 1], U32)
            nc.vector.memset(bitd[:, 0:1], 12)
            c3f8 = cpool.tile([128, 16], U32)
            nc.vector.memset(c3f8[:], 0x3F800000)
            sc2048 = cpool.tile([128, 1], F32)
            nc.vector.memset(sc2048[:], 2048.0)
            c2048 = cpool.tile([128, 16], F32)
            nc.vector.memset(c2048[:], 2048.0)

            # ---------- DVE uop warmups (first use of an ALU pair is slow) ----
            wu = cpool.tile([128, 16], U32)
            wuf = cpool.tile([128, 16], F16)
            wmx0 = cpool.tile([128, 16], F32)
            nc.vector.memset(wmx0[:], 1.0)
            wuo = cpool.tile([128, 16], U32)
            nc.vector.memset(wu[:], 1)
            nc.vector.scalar_tensor_tensor(wuo[:], wu[:], bitc[:, 0:1], wu[:],
                                           ALU.bitwise_and, ALU.bitwise_or)
            nc.vector.scalar_tensor_tensor(wuo[:], wu[:], bitd[:, 0:1], wu[:],
                                           ALU.logical_shift_left,
                                           ALU.bitwise_or)
            nc.vector.scalar_tensor_tensor(wuf[:], wmx0[:], sc2048[:, 0:1],
                                           wmx0[:], ALU.mult, ALU.subtract)
            wf2 = cpool.tile([128, 16], F16)
            nc.vector.memset(wf2[:], 1.0)
            nc.vector.tensor_tensor(wf2[:, 0:8], wf2[:, 0:8], wf2[:, 8:16],
                                    ALU.subtract)
            nc.vector.tensor_tensor(wf2[:, 0:8], wf2[:, 0:8], wf2[:, 8:16],
                                    ALU.mult)
            nc.vector.tensor_tensor(wf2[:, 0:8], wf2[:, 0:8], wf2[:, 8:16],
                                    ALU.add)
            nc.vector.tensor_scalar_max(wf2[:], wf2[:], 0.0)
            wmx = cpool.tile([128, 16], F32)
            wmp = cpool.tile([128, 8], U32)
            nc.vector.memset(wmx[:], 0.0)
            nc.vector.max(wmx[:, 0:8], wmx[:])
            nc.vector.max_index(wmp[:], wmx[:, 0:8], wmx[:])
            nc.vector.match_replace(wmx[:], wmx[:, 0:8], wmx[:], -3e38)
            nc.vector.scalar_tensor_tensor(wuo[:], wu[:], bitc[:, 2:3], wu[:],
                                           ALU.bitwise_and,
                                           ALU.logical_shift_left)
            nc.vector.scalar_tensor_tensor(wuo[:], wu[:], bitc[:, 1:2], wu[:],
                                           ALU.bitwise_and, ALU.bitwise_or)
            wms = cpool.tile([128, 16], F32)
            nc.vector.memset(wms[:], 1.0)
            nc.vector.scalar_tensor_tensor(wms[:, 0:8], wms[:, 0:8], eg,
                                           wms[:, 8:16], ALU.add, ALU.add)
            nc.vector.scalar_tensor_tensor(wms[:, 0:8], wms[:, 0:8], egv,
                                           wms[:, 8:16], ALU.add, ALU.mult)
            nc.vector.reciprocal(wms[:], wms[:])

            # ---------- tiles for prep projections + DRAM table ----------
            kT = prep.tile([128, N], F16)
            vT = prep.tile([128, N], F16)
            gT = prep.tile([128, N], F16)
            uT = prep.tile([128, NQC], F16)
            table = dpool.tile([N, ROWF], F16)

            gath_tiles = {}

            def issue_topk(gb):
                """PE dist matmuls (512-col chunks) + DVE pack/top-16."""
                wif_list = []
                for t2 in range(2):
                    t = gb * 2 + t2
                    qs = slice(t * QTILE, (t + 1) * QTILE)
                    spk = work.tile([128, N], U32, tag="spk", bufs=1)
                    for col in range(4):
                        cs = slice(col * 512, (col + 1) * 512)
                        sp = psk.tile([128, 512], F32, tag="dist")
                        nc.tensor.matmul(sp[:], qx4_s[:, qs], xt4_s[:, cs],
                                         start=True, stop=True)
                        nc.vector.scalar_tensor_tensor(
                            spk[:, cs], sp[:].bitcast(U32), bitc[:, 0:1],
                            iota[:].bitcast(U32),
                            ALU.bitwise_and, ALU.bitwise_or)
                    spkf = spk[:].bitcast(F32)
                    cand = work.tile([128, NCHUNK * 8], F32, tag="cand")
                    for c in range(NCHUNK):
                        nc.vector.max(cand[:, c * 8:(c + 1) * 8],
                                      spkf[:, c * CHUNK:(c + 1) * CHUNK])
                    winners = work.tile([128, 16], F32, tag="win")
                    pos = work.tile([128, 16], U32, tag="pos")
                    nc.vector.max(winners[:, 0:8], cand[:])
                    nc.vector.max_index(pos[:, 0:8], winners[:, 0:8], cand[:])
                    nc.vector.match_replace(cand[:], winners[:, 0:8], cand[:], -3e38)
                    nc.vector.max(winners[:, 8:16], cand[:])
                    nc.vector.max_index(pos[:, 8:16], winners[:, 8:16], cand[:])
                    # global idx = (packed & 0x7F) + (pos//8)*128, all in STT
                    # form (plain tensor_scalar is slow in this context)
                    base = work.tile([128, 16], U32, tag="base")
                    nc.vector.scalar_tensor_tensor(
                        base[:], pos[:], bitc[:, 2:3], sh16[:],
                        ALU.bitwise_and, ALU.logical_shift_left)
                    # base is a multiple of 128 and (win & 0x7F) < 128: OR == ADD
                    wid = work.tile([128, 16], U32, tag="wid")
                    nc.vector.scalar_tensor_tensor(
                        wid[:], winners[:].bitcast(U32), bitc[:, 1:2], base[:],
                        ALU.bitwise_and, ALU.bitwise_or)
                    # float(wid) without int->float cast: OR wid<<12 into
                    # 1.0f's mantissa (val = 1 + wid/2048), then *2048 - 2048
                    wid2 = work.tile([128, 16], U32, tag="wid2")
                    nc.vector.scalar_tensor_tensor(
                        wid2[:], wid[:], bitd[:, 0:1], c3f8[:],
                        ALU.logical_shift_left, ALU.bitwise_or)
                    wif = work.tile([128, 16], F16, tag="wif")
                    nc.vector.scalar_tensor_tensor(
                        wif[:], wid2[:].bitcast(F32), sc2048[:, 0:1], c2048[:],
                        ALU.mult, ALU.subtract)
                    wif_list.append(wif)
                return wif_list

            def issue_sel_gather(gb, wif_list):
                psel = pss.tile([128, 256], F32, tag="sel", bufs=1)
                for t2 in range(2):
                    rhs = work.tile([128, 256], F16, tag="rhs")
                    nc.vector.tensor_tensor(
                        rhs[:].rearrange("p (a b) -> p a b", a=16),
                        wif_list[t2][:].unsqueeze(2).broadcast_to((128, 16, 16)),
                        masks_s[:, t2 * 256:(t2 + 1) * 256].rearrange("p (a b) -> p a b", a=16),
                        ALU.mult)
                    nc.tensor.matmul(psel[:], esel_s[:], rhs[:],
                                     start=(t2 == 0), stop=(t2 == 1))
                idxs = work.tile([128, 256], I16, tag="idxs")
                nc.scalar.copy(idxs[:], psel[:])
                gath = work.tile([128, 3, 4096], F16, tag="gath", bufs=3)
                nc.gpsimd.dma_gather(gath[:], table[:], idxs[:],
                                     num_idxs=4096, num_idxs_reg=4096,
                                     elem_size=ROWF, transpose=True,
                                     single_packet=False)
                gath_tiles[gb] = gath

            def issue_mlp(gb):
                gath = gath_tiles.pop(gb)
                k3 = gath[:, 0, :].rearrange("p (a b) -> p a b", a=16)
                v3 = gath[:, 1, :].rearrange("p (a b) -> p a b", a=16)
                g3 = gath[:, 2, :].rearrange("p (a b) -> p a b", a=16)
                ub = uT[:, gb * QBLK:(gb + 1) * QBLK].unsqueeze(1) \
                    .broadcast_to((128, 16, QBLK))

                # --- h = relu(u - g) ---
                hpre = mlp.tile([128, 4096], F16, tag="ma")
                h3 = hpre[:].rearrange("p (a b) -> p a b", a=16)
                nc.vector.tensor_tensor(h3, ub, g3, ALU.subtract)
                nc.scalar.activation(hpre[:], hpre[:], ACTF.Relu)
                # --- pos = W_d2 @ h + b_d2 ---
                pos = mlp.tile([128, 4096], F16, tag="mb")
                for col in range(8):
                    cs = slice(col * 512, (col + 1) * 512)
                    pm = psm.tile([128, 512], F32, tag="mm")
                    nc.tensor.matmul(pm[:], wd2_s[:], hpre[:, cs], start=True, stop=True)
                    nc.scalar.add(pos[:, cs], pm[:], b_d2)
                pos3 = pos[:].rearrange("p (a b) -> p a b", a=16)

                # --- g1pre = pos - k  (q_attn folded into b_g1') ---
                g1pre = mlp.tile([128, 4096], F16, tag="mc")
                nc.vector.tensor_tensor(
                    g1pre[:].rearrange("p (a b) -> p a b", a=16),
                    pos3, k3, ALU.subtract)

                # --- vpos = v + pos (early: frees the gath buffer) ---
                vpos = mlp.tile([128, 4096], F16, tag="mv")
                vp3 = vpos[:].rearrange("p (a b) -> p a b", a=16)
                nc.vector.tensor_tensor(vp3, v3, pos3, ALU.add)

                # --- g1 = relu(W_g1 @ g1pre + b_g1') ---
                g1 = mlp.tile([128, 4096], F16, tag="ma")
                for col in range(8):
                    cs = slice(col * 512, (col + 1) * 512)
                    pm = psm.tile([128, 512], F32, tag="mm")
                    nc.tensor.matmul(pm[:], wg1_s[:], g1pre[:, cs], start=True, stop=True)
                    nc.scalar.activation(g1[:, cs], pm[:], ACTF.Relu, bias=b_g1)

                # --- expt = exp(W_g2 @ g1 + b_g2) ---
                expt = mlp.tile([128, 4096], F16, tag="md")
                for col in range(8):
                    cs = slice(col * 512, (col + 1) * 512)
                    pm = psm.tile([128, 512], F32, tag="mm")
                    nc.tensor.matmul(pm[:], wg2_s[:], g1[:, cs], start=True, stop=True)
                    nc.scalar.activation(expt[:, cs], pm[:], ACTF.Exp, bias=b_g2)
                e3 = expt[:].rearrange("p (a b) -> p a b", a=16)

                # --- esum tree (f16 halves; eg folded via STT) ---
                e8 = mlp.tile([128, 8, QBLK], F16, tag="t8")
                nc.vector.tensor_tensor(e8[:], e3[:, 0:8, :], e3[:, 8:16, :], ALU.add)
                e4 = mlp.tile([128, 4, QBLK], F16, tag="t4")
                nc.vector.tensor_tensor(e4[:], e8[:, 0:4, :], e8[:, 4:8, :], ALU.add)
                e2 = mlp.tile([128, 2, QBLK], F16, tag="t2")
                nc.vector.tensor_tensor(e2[:], e4[:, 0:2, :], e4[:, 2:4, :], ALU.add)
                esum = mlp.tile([128, QBLK], F32, tag="es")
                nc.vector.scalar_tensor_tensor(esum[:], e2[:, 0, :], eg,
                                               e2[:, 1, :], ALU.add, ALU.add)
                rec = mlp.tile([128, QBLK], F32, tag="rc")
                nc.vector.reciprocal(rec[:], esum[:])

                # --- wprod = expt * vpos ; wsum tree ---
                wp = mlp.tile([128, 4096], F16, tag="ma")
                wp3 = wp[:].rearrange("p (a b) -> p a b", a=16)
                nc.vector.tensor_tensor(wp3, e3, vp3, ALU.mult)
                w8 = mlp.tile([128, 8, QBLK], F16, tag="t8")
                nc.vector.tensor_tensor(w8[:], wp3[:, 0:8, :], wp3[:, 8:16, :], ALU.add)
                w4 = mlp.tile([128, 4, QBLK], F16, tag="t4")
                nc.vector.tensor_tensor(w4[:], w8[:, 0:4, :], w8[:, 4:8, :], ALU.add)
                w2 = mlp.tile([128, 2, QBLK], F16, tag="t2")
                nc.vector.tensor_tensor(w2[:], w4[:, 0:2, :], w4[:, 2:4, :], ALU.add)
                wsum = mlp.tile([128, QBLK], F32, tag="ws")
                nc.vector.tensor_tensor(wsum[:], w2[:, 0, :], w2[:, 1, :], ALU.add)

                # --- res = (wsum + egv) * (1 / esum) ---
                res = mlp.tile([128, QBLK], F32, tag="res")
                nc.vector.scalar_tensor_tensor(res[:], wsum[:], egv, rec[:],
                                               ALU.add, ALU.mult)

                # --- transpose out and store ---
                for t2 in range(2):
                    po = pss.tile([128, 128], F32, tag="small")
                    nc.tensor.transpose(po[:], res[:, t2 * 128:(t2 + 1) * 128], id32[:])
                    osb = work.tile([128, 128], F32, tag="osb")
                    nc.scalar.copy(osb[:], po[:])
                    nc.sync.dma_start(
                        out[gb * QBLK + t2 * 128: gb * QBLK + (t2 + 1) * 128, :],
                        osb[:])

            # topk(0) first so its dist matmuls/DVE chain overlap the prep
            wif0 = issue_topk(0)

            # prep projections per 512-col group, table chunks right after
            for col in range(4):
                cs = slice(col * 512, (col + 1) * 512)
                acc_k = psm.tile([128, 512], F32, tag="mm")
                nc.tensor.matmul(acc_k[:], wk_s[:, 0:DIM],
                                 pts_s[:, col * 512:(col + 1) * 512],
                                 start=True, stop=False)
                nc.tensor.matmul(acc_k[:], wk_s[:, DIM:2 * DIM],
                                 pts_s[:, N + col * 512:N + (col + 1) * 512],
                                 start=False, stop=True)
                nc.scalar.copy(kT[:, cs], acc_k[:])
                acc_v = psm.tile([128, 512], F32, tag="mm")
                nc.tensor.matmul(acc_v[:], wv_s[:, 0:DIM],
                                 pts_s[:, col * 512:(col + 1) * 512],
                                 start=True, stop=False)
                nc.tensor.matmul(acc_v[:], wv_s[:, DIM:2 * DIM],
                                 pts_s[:, N + col * 512:N + (col + 1) * 512],
                                 start=False, stop=True)
                nc.scalar.copy(vT[:, cs], acc_v[:])
                acc_g = psm.tile([128, 512], F32, tag="mm")
                nc.tensor.matmul(acc_g[:], wd1_s[:], xyzn4_s[:, cs], start=True, stop=True)
                nc.scalar.copy(gT[:, cs], acc_g[:])
                for c4 in range(4):
                    c = col * 4 + c4
                    rs = slice(c * 128, (c + 1) * 128)
                    row_sb = work.tile([128, ROWF], F16, tag="rowsb")
                    for j, srcT in enumerate((kT, vT, gT)):
                        pt = pss.tile([128, 128], F16, tag="small")
                        nc.tensor.transpose(pt[:], srcT[:, rs], id16[:])
                        nc.scalar.copy(row_sb[:, j * 128:(j + 1) * 128], pt[:])
                    nc.sync.dma_start(table[rs, :], row_sb[:])

            issue_sel_gather(0, wif0)
            # u = W_d1 @ xyz_q + b (off gather(0)'s critical path)
            for col in range(4):
                cs = slice(col * 512, (col + 1) * 512)
                acc_u = psm.tile([128, 512], F32, tag="mm")
                nc.tensor.matmul(acc_u[:], wd1_s[:], xyzq4_s[:, cs],
                                 start=True, stop=True)
                nc.scalar.copy(uT[:, cs], acc_u[:])

            for it in range(1, NBLK + 2):
                if it < NBLK:
                    wif_list = issue_topk(it)
                    issue_sel_gather(it, wif_list)
                if it >= 2:
                    issue_mlp(it - 2)

    nc.compile()
    return nc


def _host_prep(inputs):
    """Build the 8 per-core input maps from full inputs (layout prep only)."""
    xyz_q = np.asarray(inputs["xyz_q"], np.float32)
    lat_rep = np.asarray(inputs["lat_rep"], np.float32)
    xyz = np.asarray(inputs["xyz"], np.float32)
    points = np.asarray(inputs["points"], np.float32)
    W_d1 = np.asarray(inputs["W_d1"], np.float32); b_d1 = np.asarray(inputs["b_d1"], np.float32)
    W_d2 = np.asarray(inputs["W_d2"], np.float32); b_d2 = np.asarray(inputs["b_d2"], np.float32)
    W_g1 = np.asarray(inputs["W_g1"], np.float32); b_g1 = np.asarray(inputs["b_g1"], np.float32)
    W_g2 = np.asarray(inputs["W_g2"], np.float32); b_g2 = np.asarray(inputs["b_g2"], np.float32)
    W_kg = np.asarray(inputs["W_kg"], np.float32)
    W_vg = np.asarray(inputs["W_vg"], np.float32)
    W_q = np.asarray(inputs["W_q"], np.float32)
    W_k = np.asarray(inputs["W_k"], np.float32)
    W_v = np.asarray(inputs["W_v"], np.float32)

    # per-batch global-slot constants
    q_attn = lat_rep @ W_q.T                      # [B, DIM]
    k_g = lat_rep @ W_kg.T
    v_g = lat_rep @ W_vg.T
    tg = q_attn - k_g
    g1g = np.maximum(tg @ W_g1.T + b_g1, 0.0)
    logit_g = g1g @ W_g2.T + b_g2
    exp_g = np.exp(logit_g)                       # [B, DIM]
    egv = exp_g * v_g
    # fold q_attn into the g1 bias: g1 = relu(W_g1 @ (pos - k) + b_g1')
    b_g1p = b_g1[None, :] + q_attn @ W_g1.T       # [B, DIM]

    # constants
    qp = np.arange(128)
    esel = (qp[:, None] % 16 == qp[None, :] % 16).astype(np.float16)  # [q',p]
    masks = np.zeros((2, 128, 256), np.float16)
    g_of = qp // 16                               # q' // 16 in 0..7
    for t in range(2):
        for nb in range(16):
            for g in range(16):
                masks[t, :, nb * 16 + g] = (g_of == (g - t * 8)).astype(np.float16)
    ident16 = np.eye(128, dtype=np.float16)
    ident32 = np.eye(128, dtype=np.float32)
    iota = np.tile(np.tile(np.arange(CHUNK, dtype=np.int32), 2), (128, 1))

    wd1_l = np.concatenate([W_d1.T, b_d1[None, :]], axis=0).astype(np.float16)  # [4,128]

    maps = []
    for core in range(8):
        b, h = core // 2, core % 2
        qsl = slice(h * NQC, (h + 1) * NQC)
        xq = xyz_q[b, qsl]                        # [2048, 3]
        xn = xyz[b]                               # [2048, 3]
        qx4 = np.concatenate([2.0 * xq.T, np.ones((1, NQC), np.float32)], axis=0)
        xt4 = np.concatenate([xn.T, -np.sum(xn * xn, axis=1)[None, :]], axis=0)
        xyzq4 = np.concatenate([xq.T, np.ones((1, NQC), np.float32)], axis=0).astype(np.float16)
        xyzn4 = np.concatenate([xn.T, np.zeros((1, N), np.float32)], axis=0).astype(np.float16)
        pT = points[b].T.astype(np.float16)          # [256, N]
        ptsT = np.concatenate([pT[0:128], pT[128:256]], axis=1)  # [128, 2N]
        colv = np.stack([b_d2, b_g1p[b], b_g2, exp_g[b], egv[b]],
                        axis=1).astype(np.float32)
        maps.append({
            "qx4": np.ascontiguousarray(qx4, np.float32),
            "xt4": np.ascontiguousarray(xt4, np.float32),
            "ptsT": np.ascontiguousarray(ptsT),
            "xyzq4": np.ascontiguousarray(xyzq4),
            "xyzn4": np.ascontiguousarray(xyzn4),
            "wk_l": np.ascontiguousarray(np.concatenate(
                [W_k.T[0:128], W_k.T[128:256]], axis=1).astype(np.float16)),
            "wv_l": np.ascontiguousarray(np.concatenate(
                [W_v.T[0:128], W_v.T[128:256]], axis=1).astype(np.float16)),
            "wd1_l": np.ascontiguousarray(wd1_l),
            "wd2_l": np.ascontiguousarray(W_d2.T.astype(np.float16)),
            "wg1_l": np.ascontiguousarray(W_g1.T.astype(np.float16)),
            "wg2_l": np.ascontiguousarray(W_g2.T.astype(np.float16)),
            "colv": np.ascontiguousarray(colv),
            "esel": np.ascontiguousarray(esel),
            "masks": np.ascontiguousarray(
                np.concatenate([masks[0], masks[1]], axis=1)),
            "ident16": ident16,
            "ident32": ident32,
            "iota_d": iota,
        })
    return maps


def kernel(**inputs):
    if "nc" not in _CACHE:
        _CACHE["nc"] = _build()
    nc = _CACHE["nc"]
    maps = _host_prep(inputs)
    res = run_bass_kernel_spmd(nc, maps, core_ids=list(range(8)))
    out = np.empty((B, NQ, DIM), np.float32)
    for core in range(8):
        b, h = core // 2, core % 2
        out[b, h * NQC:(h + 1) * NQC, :] = res.results[core]["out"]
    return out


# revision 17
# speedup vs baseline: 1.0076x; 1.0076x over previous
"""Trainium2 Bass kernel for nn_CrossTransformerBlock (sparse kNN cross-attention).

Shapes (hardcoded): B=4, NQ=4096, N=2048, DIM=128, DG=256, DI=256, K=16.
Sharding: 8 cores = (batch b, query-half h); each core handles 2048 queries
against its batch's 2048 points.

Per-core design (software-pipelined over 8 blocks of 256 queries):
  prep:  PE projects k_all/v_all (points @ W), g_all = W_d1@xyz into a
         row-major DRAM table [2048, 384] f16 (k|v|g); u deferred.
  topk:  PE computes s = 2 q.x - |x|^2 per 512-col chunk; DVE ORs the
         GLOBAL 11-bit column index into the mantissa low bits, then
         per-128-chunk top-8 (max8) + match_replace merge -> top-16 packed;
         idx = value & 0x7FF (one STT; no max_index needed).
  gather: selector matmul -> wrapped i16 layout; dma_gather(transpose=True)
         pulls 16 rows/query feature-major; 32.4us/block on GpSimd = the
         pipeline bottleneck; issue order keeps gathers back-to-back.
  mlp:   h=relu(u-g); pos=W_d2 h+b; g1=relu(W_g1(pos-k)+b_g1') with
         q_attn folded into b_g1' on host; exp on ACT evac; softmax sums
         via f16 tree adds (eg folded via STT); global slot via eg/egv.
  Loop issue order per iteration: topk(it) -> sel+gather(it) -> mlp(it-1),
  so the gather-gating chain never sits behind gather-dependent MLP ops
  in the in-order per-engine queues.
"""

import numpy as np

import concourse.bass as bass
import concourse.bacc as bacc
import concourse.mybir as mybir
from concourse.tile import TileContext
from concourse.bass_utils import run_bass_kernel_spmd

F32 = mybir.dt.float32
F16 = mybir.dt.float16
U32 = mybir.dt.uint32
I32 = mybir.dt.int32
I16 = mybir.dt.int16
ALU = mybir.AluOpType
ACTF = mybir.ActivationFunctionType

B, NQ, N, DIM, DG, DI, K = 4, 4096, 2048, 128, 256, 256, 16
NQC = 2048          # queries per core
QTILE = 128         # topk tile (queries on partitions)
QBLK = 256          # gather/MLP block
NBLK = NQC // QBLK
CHUNK = 256         # topk candidate chunk (top-8 per chunk)
NCHUNK = N // CHUNK
ROWF = 3 * DIM      # table row features (k|v|g)

_CACHE = {}


def _build():
    nc = bacc.Bacc("TRN2", target_bir_lowering=False, debug=False, num_devices=8)

    # ---- external inputs (per core) ----
    qx4 = nc.dram_tensor("qx4", [4, NQC], F32, kind="ExternalInput")
    xt4 = nc.dram_tensor("xt4", [4, N], F32, kind="ExternalInput")
    ptsT = nc.dram_tensor("ptsT", [128, 2 * N], F16, kind="ExternalInput")
    xyzq4 = nc.dram_tensor("xyzq4", [4, NQC], F16, kind="ExternalInput")
    xyzn4 = nc.dram_tensor("xyzn4", [4, N], F16, kind="ExternalInput")
    wk_l = nc.dram_tensor("wk_l", [128, 2 * DIM], F16, kind="ExternalInput")
    wv_l = nc.dram_tensor("wv_l", [128, 2 * DIM], F16, kind="ExternalInput")
    wd1_l = nc.dram_tensor("wd1_l", [4, DIM], F16, kind="ExternalInput")
    wd2_l = nc.dram_tensor("wd2_l", [DIM, DIM], F16, kind="ExternalInput")
    wg1_l = nc.dram_tensor("wg1_l", [DIM, DIM], F16, kind="ExternalInput")
    wg2_l = nc.dram_tensor("wg2_l", [DIM, DIM], F16, kind="ExternalInput")
    # per-partition column vectors [128, 5] f32: b_d2, b_g1', b_g2, eg, egv
    colv = nc.dram_tensor("colv", [DIM, 5], F32, kind="ExternalInput")
    # constants
    esel = nc.dram_tensor("esel", [128, 128], F16, kind="ExternalInput")
    masks = nc.dram_tensor("masks", [128, 2 * 256], F16, kind="ExternalInput")
    ident16 = nc.dram_tensor("ident16", [128, 128], F16, kind="ExternalInput")
    ident32 = nc.dram_tensor("ident32", [128, 128], F32, kind="ExternalInput")
    iota_d = nc.dram_tensor("iota_d", [128, 512], I32, kind="ExternalInput")

    out = nc.dram_tensor("out", [NQC, DIM], F32, kind="ExternalOutput")

    with TileContext(nc) as tc:
        with tc.tile_pool(name="const", bufs=1) as cpool, \
             tc.tile_pool(name="prep", bufs=1) as prep, \
             tc.tile_pool(name="work", bufs=2) as work, \
             tc.tile_pool(name="mlp", bufs=1) as mlp, \
             tc.tile_pool(name="psk", bufs=2, space="PSUM") as psk, \
             tc.tile_pool(name="psm", bufs=3, space="PSUM") as psm, \
             tc.tile_pool(name="pss", bufs=2, space="PSUM") as pss, \
             tc.tile_pool(name="dram", bufs=1, space="DRAM") as dpool:

            # ---------- load operands (order = need order) ----------
            qx4_s = cpool.tile([4, NQC], F32)
            xt4_s = cpool.tile([4, N], F32)
            nc.sync.dma_start(qx4_s[:], qx4[:])
            nc.sync.dma_start(xt4_s[:], xt4[:])
            pts_s = cpool.tile([128, 2 * N], F16)
            nc.sync.dma_start(pts_s[:], ptsT[:])
            xyzn4_s = cpool.tile([4, N], F16)
            nc.sync.dma_start(xyzn4_s[:], xyzn4[:])
            wk_s = cpool.tile([128, 2 * DIM], F16)
            wv_s = cpool.tile([128, 2 * DIM], F16)
            nc.sync.dma_start(wk_s[:], wk_l[:])
            nc.sync.dma_start(wv_s[:], wv_l[:])
            wd1_s = cpool.tile([4, DIM], F16)
            nc.sync.dma_start(wd1_s[:], wd1_l[:])
            id16 = cpool.tile([128, 128], F16)
            nc.sync.dma_start(id16[:], ident16[:])
            iota = cpool.tile([128, 512], I32)
            nc.sync.dma_start(iota[:], iota_d[:])
            esel_s = cpool.tile([128, 128], F16)
            nc.sync.dma_start(esel_s[:], esel[:])
            masks_s = cpool.tile([128, 2 * 256], F16)
            nc.sync.dma_start(masks_s[:], masks[:])
            xyzq4_s = cpool.tile([4, NQC], F16)
            nc.sync.dma_start(xyzq4_s[:], xyzq4[:])
            wd2_s = cpool.tile([DIM, DIM], F16)
            wg1_s = cpool.tile([DIM, DIM], F16)
            wg2_s = cpool.tile([DIM, DIM], F16)
            nc.sync.dma_start(wd2_s[:], wd2_l[:])
            nc.sync.dma_start(wg1_s[:], wg1_l[:])
            nc.sync.dma_start(wg2_s[:], wg2_l[:])
            colv_s = cpool.tile([DIM, 5], F32)
            nc.sync.dma_start(colv_s[:], colv[:])
            b_d2 = colv_s[:, 0:1]
            b_g1 = colv_s[:, 1:2]
            b_g2 = colv_s[:, 2:3]
            eg = colv_s[:, 3:4]
            egv = colv_s[:, 4:5]
            id32 = cpool.tile([128, 128], F32)
            nc.sync.dma_start(id32[:], ident32[:])

            # pack mask / idx mask columns + shift counts for STT forms
            bitc = cpool.tile([128, 3], U32)
            nc.vector.memset(bitc[:, 0:1], 0xFFFFFF00)
            nc.vector.memset(bitc[:, 1:2], 0xFF)
            nc.vector.memset(bitc[:, 2:3], 0x78)
            sh16 = cpool.tile([128, 16], U32)
            nc.vector.memset(sh16[:], 5)
            bitd = cpool.tile([128, 1], U32)
            nc.vector.memset(bitd[:, 0:1], 12)
            c3f8 = cpool.tile([128, 16], U32)
            nc.vector.memset(c3f8[:], 0x3F800000)
            sc2048 = cpool.tile([128, 1], F32)
            nc.vector.memset(sc2048[:], 2048.0)
            c2048 = cpool.tile([128, 16], F32)
            nc.vector.memset(c2048[:], 2048.0)

            # ---------- DVE uop warmups (first use of an ALU pair is slow) ----
            wu = cpool.tile([128, 16], U32)
            wuf = cpool.tile([128, 16], F16)
            wmx0 = cpool.tile([128, 16], F32)
            nc.vector.memset(wmx0[:], 1.0)
            wuo = cpool.tile([128, 16], U32)
            nc.vector.memset(wu[:], 1)
            nc.vector.scalar_tensor_tensor(wuo[:], wu[:], bitc[:, 0:1], wu[:],
                                           ALU.bitwise_and, ALU.bitwise_or)
            nc.vector.scalar_tensor_tensor(wuo[:], wu[:], bitd[:, 0:1], wu[:],
                                           ALU.logical_shift_left,
                                           ALU.bitwise_or)
            nc.vector.scalar_tensor_tensor(wuf[:], wmx0[:], sc2048[:, 0:1],
                                           wmx0[:], ALU.mult, ALU.subtract)
            wf2 = cpool.tile([128, 16], F16)
            nc.vector.memset(wf2[:], 1.0)
            nc.vector.tensor_tensor(wf2[:, 0:8], wf2[:, 0:8], wf2[:, 8:16],
                                    ALU.subtract)
            nc.vector.tensor_tensor(wf2[:, 0:8], wf2[:, 0:8], wf2[:, 8:16],
                                    ALU.mult)
            nc.vector.tensor_tensor(wf2[:, 0:8], wf2[:, 0:8], wf2[:, 8:16],
                                    ALU.add)
            nc.vector.tensor_scalar_max(wf2[:], wf2[:], 0.0)
            wmx = cpool.tile([128, 16], F32)
            wmp = cpool.tile([128, 8], U32)
            nc.vector.memset(wmx[:], 0.0)
            nc.vector.max(wmx[:, 0:8], wmx[:])
            nc.vector.max_index(wmp[:], wmx[:, 0:8], wmx[:])
            nc.vector.match_replace(wmx[:], wmx[:, 0:8], wmx[:], -3e38)
            nc.vector.scalar_tensor_tensor(wuo[:], wu[:], bitc[:, 2:3], wu[:],
                                           ALU.bitwise_and,
                                           ALU.logical_shift_left)
            nc.vector.scalar_tensor_tensor(wuo[:], wu[:], bitc[:, 1:2], wu[:],
                                           ALU.bitwise_and, ALU.bitwise_or)
            wms = cpool.tile([128, 16], F32)
            nc.vector.memset(wms[:], 1.0)
            nc.vector.scalar_tensor_tensor(wms[:, 0:8], wms[:, 0:8], eg,
                                           wms[:, 8:16], ALU.add, ALU.add)
            nc.vector.scalar_tensor_tensor(wms[:, 0:8], wms[:, 0:8], egv,
                                           wms[:, 8:16], ALU.add, ALU.mult)
            nc.vector.reciprocal(wms[:], wms[:])

            # ---------- tiles for prep projections + DRAM table ----------
            kT = prep.tile([128, N], F16)
            vT = prep.tile([128, N], F16)
            gT = prep.tile([128, N], F16)
            uT = prep.tile([128, NQC], F16)
            table = dpool.tile([N, ROWF], F16)

            gath_tiles = {}

            def issue_topk(gb):
                """PE dist matmuls (512-col chunks) + DVE pack/top-16."""
                wif_list = []
                for t2 in range(2):
                    t = gb * 2 + t2
                    qs = slice(t * QTILE, (t + 1) * QTILE)
                    spk = work.tile([128, N], U32, tag="spk", bufs=1)
                    for col in range(4):
                        cs = slice(col * 512, (col + 1) * 512)
                        sp = psk.tile([128, 512], F32, tag="dist")
                        nc.tensor.matmul(sp[:], qx4_s[:, qs], xt4_s[:, cs],
                                         start=True, stop=True)
                        nc.vector.scalar_tensor_tensor(
                            spk[:, cs], sp[:].bitcast(U32), bitc[:, 0:1],
                            iota[:].bitcast(U32),
                            ALU.bitwise_and, ALU.bitwise_or)
                    spkf = spk[:].bitcast(F32)
                    cand = work.tile([128, NCHUNK * 8], F32, tag="cand")
                    for c in range(NCHUNK):
                        nc.vector.max(cand[:, c * 8:(c + 1) * 8],
                                      spkf[:, c * CHUNK:(c + 1) * CHUNK])
                    winners = work.tile([128, 16], F32, tag="win")
                    pos = work.tile([128, 16], U32, tag="pos")
                    nc.vector.max(winners[:, 0:8], cand[:])
                    nc.vector.max_index(pos[:, 0:8], winners[:, 0:8], cand[:])
                    nc.vector.match_replace(cand[:], winners[:, 0:8], cand[:], -3e38)
                    nc.vector.max(winners[:, 8:16], cand[:])
                    nc.vector.max_index(pos[:, 8:16], winners[:, 8:16], cand[:])
                    # global idx = (packed & 0x7F) + (pos//8)*128, all in STT
                    # form (plain tensor_scalar is slow in this context)
                    base = work.tile([128, 16], U32, tag="base")
                    nc.vector.scalar_tensor_tensor(
                        base[:], pos[:], bitc[:, 2:3], sh16[:],
                        ALU.bitwise_and, ALU.logical_shift_left)
                    # base is a multiple of 128 and (win & 0x7F) < 128: OR == ADD
                    wid = work.tile([128, 16], U32, tag="wid")
                    nc.vector.scalar_tensor_tensor(
                        wid[:], winners[:].bitcast(U32), bitc[:, 1:2], base[:],
                        ALU.bitwise_and, ALU.bitwise_or)
                    # float(wid) without int->float cast: OR wid<<12 into
                    # 1.0f's mantissa (val = 1 + wid/2048), then *2048 - 2048
                    wid2 = work.tile([128, 16], U32, tag="wid2")
                    nc.vector.scalar_tensor_tensor(
                        wid2[:], wid[:], bitd[:, 0:1], c3f8[:],
                        ALU.logical_shift_left, ALU.bitwise_or)
                    wif = work.tile([128, 16], F16, tag="wif")
                    nc.vector.scalar_tensor_tensor(
                        wif[:], wid2[:].bitcast(F32), sc2048[:, 0:1], c2048[:],
                        ALU.mult, ALU.subtract)
                    wif_list.append(wif)
                return wif_list

            def issue_sel_gather(gb, wif_list):
                psel = pss.tile([128, 256], F32, tag="sel", bufs=1)
                for t2 in range(2):
                    rhs = work.tile([128, 256], F16, tag="rhs")
                    nc.vector.tensor_tensor(
                        rhs[:].rearrange("p (a b) -> p a b", a=16),
                        wif_list[t2][:].unsqueeze(2).broadcast_to((128, 16, 16)),
                        masks_s[:, t2 * 256:(t2 + 1) * 256].rearrange("p (a b) -> p a b", a=16),
                        ALU.mult)
                    nc.tensor.matmul(psel[:], esel_s[:], rhs[:],
                                     start=(t2 == 0), stop=(t2 == 1))
                idxs = work.tile([128, 256], I16, tag="idxs")
                nc.scalar.copy(idxs[:], psel[:])
                gath = work.tile([128, 3, 4096], F16, tag=f"gath{gb % 3}",
                                 bufs=1)
                nc.gpsimd.dma_gather(gath[:], table[:], idxs[:],
                                     num_idxs=4096, num_idxs_reg=4096,
                                     elem_size=ROWF, transpose=True,
                                     single_packet=False)
                gath_tiles[gb] = gath

            def issue_mlp(gb):
                gath = gath_tiles.pop(gb)
                k3 = gath[:, 0, :].rearrange("p (a b) -> p a b", a=16)
                v3 = gath[:, 1, :].rearrange("p (a b) -> p a b", a=16)
                g3 = gath[:, 2, :].rearrange("p (a b) -> p a b", a=16)
                ub = uT[:, gb * QBLK:(gb + 1) * QBLK].unsqueeze(1) \
                    .broadcast_to((128, 16, QBLK))

                # --- h = relu(u - g) ---
                hpre = mlp.tile([128, 4096], F16, tag="ma")
                h3 = hpre[:].rearrange("p (a b) -> p a b", a=16)
                nc.vector.tensor_tensor(h3, ub, g3, ALU.subtract)
                nc.scalar.activation(hpre[:], hpre[:], ACTF.Relu)
                # --- pos = W_d2 @ h + b_d2 ---
                pos = mlp.tile([128, 4096], F16, tag="mb")
                for col in range(8):
                    cs = slice(col * 512, (col + 1) * 512)
                    pm = psm.tile([128, 512], F32, tag="mm")
                    nc.tensor.matmul(pm[:], wd2_s[:], hpre[:, cs], start=True, stop=True)
                    nc.scalar.add(pos[:, cs], pm[:], b_d2)
                pos3 = pos[:].rearrange("p (a b) -> p a b", a=16)

                # --- g1pre = pos - k  (q_attn folded into b_g1') ---
                g1pre = mlp.tile([128, 4096], F16, tag="mc")
                nc.vector.tensor_tensor(
                    g1pre[:].rearrange("p (a b) -> p a b", a=16),
                    pos3, k3, ALU.subtract)

                # --- vpos = v + pos (early: frees the gath buffer) ---
                vpos = mlp.tile([128, 4096], F16, tag="mv")
                vp3 = vpos[:].rearrange("p (a b) -> p a b", a=16)
                nc.vector.tensor_tensor(vp3, v3, pos3, ALU.add)

                # --- g1 = relu(W_g1 @ g1pre + b_g1') ---
                g1 = mlp.tile([128, 4096], F16, tag="ma")
                for col in range(8):
                    cs = slice(col * 512, (col + 1) * 512)
                    pm = psm.tile([128, 512], F32, tag="mm")
                    nc.tensor.matmul(pm[:], wg1_s[:], g1pre[:, cs], start=True, stop=True)
                    nc.scalar.activation(g1[:, cs], pm[:], ACTF.Relu, bias=b_g1)

                # --- expt = exp(W_g2 @ g1 + b_g2) ---
                expt = mlp.tile([128, 4096], F16, tag="md")
                for col in range(8):
                    cs = slice(col * 512, (col + 1) * 512)
                    pm = psm.tile([128, 512], F32, tag="mm")
                    nc.tensor.matmul(pm[:], wg2_s[:], g1[:, cs], start=True, stop=True)
                    nc.scalar.activation(expt[:, cs], pm[:], ACTF.Exp, bias=b_g2)
                e3 = expt[:].rearrange("p (a b) -> p a b", a=16)

                # --- esum tree (f16 halves; eg folded via STT) ---
                e8 = mlp.tile([128, 8, QBLK], F16, tag="t8")
                nc.vector.tensor_tensor(e8[:], e3[:, 0:8, :], e3[:, 8:16, :], ALU.add)
                e4 = mlp.tile([128, 4, QBLK], F16, tag="t4")
                nc.vector.tensor_tensor(e4[:], e8[:, 0:4, :], e8[:, 4:8, :], ALU.add)
                e2 = mlp.tile([128, 2, QBLK], F16, tag="t2")
                nc.vector.tensor_tensor(e2[:], e4[:, 0:2, :], e4[:, 2:4, :], ALU.add)
                esum = mlp.tile([128, QBLK], F32, tag="es")
                nc.vector.scalar_tensor_tensor(esum[:], e2[:, 0, :], eg,
                                               e2[:, 1, :], ALU.add, ALU.add)
                rec = mlp.tile([128, QBLK], F32, tag="rc")
                nc.vector.reciprocal(rec[:], esum[:])

                # --- wprod = expt * vpos ; wsum tree ---
                wp = mlp.tile([128, 4096], F16, tag="ma")
                wp3 = wp[:].rearrange("p (a b) -> p a b", a=16)
                nc.vector.tensor_tensor(wp3, e3, vp3, ALU.mult)
                w8 = mlp.tile([128, 8, QBLK], F16, tag="t8")
                nc.vector.tensor_tensor(w8[:], wp3[:, 0:8, :], wp3[:, 8:16, :], ALU.add)
                w4 = mlp.tile([128, 4, QBLK], F16, tag="t4")
                nc.vector.tensor_tensor(w4[:], w8[:, 0:4, :], w8[:, 4:8, :], ALU.add)
                w2 = mlp.tile([128, 2, QBLK], F16, tag="t2")
                nc.vector.tensor_tensor(w2[:], w4[:, 0:2, :], w4[:, 2:4, :], ALU.add)
                wsum = mlp.tile([128, QBLK], F32, tag="ws")
                nc.vector.tensor_tensor(wsum[:], w2[:, 0, :], w2[:, 1, :], ALU.add)

                # --- res = (wsum + egv) * (1 / esum) ---
                res = mlp.tile([128, QBLK], F32, tag="res")
                nc.vector.scalar_tensor_tensor(res[:], wsum[:], egv, rec[:],
                                               ALU.add, ALU.mult)

                # --- transpose out and store ---
                for t2 in range(2):
                    po = pss.tile([128, 128], F32, tag="small")
                    nc.tensor.transpose(po[:], res[:, t2 * 128:(t2 + 1) * 128], id32[:])
                    osb = work.tile([128, 128], F32, tag="osb")
                    nc.scalar.copy(osb[:], po[:])
                    nc.sync.dma_start(
                        out[gb * QBLK + t2 * 128: gb * QBLK + (t2 + 1) * 128, :],
                        osb[:])

            # topk(0) first so its dist matmuls/DVE chain overlap the prep
            wif0 = issue_topk(0)

            # prep projections per 512-col group, table chunks right after
            for col in range(4):
                cs = slice(col * 512, (col + 1) * 512)
                acc_k = psm.tile([128, 512], F32, tag="mm")
                nc.tensor.matmul(acc_k[:], wk_s[:, 0:DIM],
                                 pts_s[:, col * 512:(col + 1) * 512],
                                 start=True, stop=False)
                nc.tensor.matmul(acc_k[:], wk_s[:, DIM:2 * DIM],
                                 pts_s[:, N + col * 512:N + (col + 1) * 512],
                                 start=False, stop=True)
                nc.scalar.copy(kT[:, cs], acc_k[:])
                acc_v = psm.tile([128, 512], F32, tag="mm")
                nc.tensor.matmul(acc_v[:], wv_s[:, 0:DIM],
                                 pts_s[:, col * 512:(col + 1) * 512],
                                 start=True, stop=False)
                nc.tensor.matmul(acc_v[:], wv_s[:, DIM:2 * DIM],
                                 pts_s[:, N + col * 512:N + (col + 1) * 512],
                                 start=False, stop=True)
                nc.scalar.copy(vT[:, cs], acc_v[:])
                acc_g = psm.tile([128, 512], F32, tag="mm")
                nc.tensor.matmul(acc_g[:], wd1_s[:], xyzn4_s[:, cs], start=True, stop=True)
                nc.scalar.copy(gT[:, cs], acc_g[:])
                for c4 in range(4):
                    c = col * 4 + c4
                    rs = slice(c * 128, (c + 1) * 128)
                    row_sb = work.tile([128, ROWF], F16, tag="rowsb")
                    for j, srcT in enumerate((kT, vT, gT)):
                        pt = pss.tile([128, 128], F16, tag="small")
                        nc.tensor.transpose(pt[:], srcT[:, rs], id16[:])
                        nc.scalar.copy(row_sb[:, j * 128:(j + 1) * 128], pt[:])
                    nc.sync.dma_start(table[rs, :], row_sb[:])

            issue_sel_gather(0, wif0)
            # u = W_d1 @ xyz_q + b (off gather(0)'s critical path)
            for col in range(4):
                cs = slice(col * 512, (col + 1) * 512)
                acc_u = psm.tile([128, 512], F32, tag="mm")
                nc.tensor.matmul(acc_u[:], wd1_s[:], xyzq4_s[:, cs],
                                 start=True, stop=True)
                nc.scalar.copy(uT[:, cs], acc_u[:])

            for it in range(1, NBLK + 2):
                if it < NBLK:
                    wif_list = issue_topk(it)
                    issue_sel_gather(it, wif_list)
                if it >= 2:
                    issue_mlp(it - 2)

    nc.compile()
    return nc


def _host_prep(inputs):
    """Build the 8 per-core input maps from full inputs (layout prep only)."""
    xyz_q = np.asarray(inputs["xyz_q"], np.float32)
    lat_rep = np.asarray(inputs["lat_rep"], np.float32)
    xyz = np.asarray(inputs["xyz"], np.float32)
    points = np.asarray(inputs["points"], np.float32)
    W_d1 = np.asarray(inputs["W_d1"], np.float32); b_d1 = np.asarray(inputs["b_d1"], np.float32)
    W_d2 = np.asarray(inputs["W_d2"], np.float32); b_d2 = np.asarray(inputs["b_d2"], np.float32)
    W_g1 = np.asarray(inputs["W_g1"], np.float32); b_g1 = np.asarray(inputs["b_g1"], np.float32)
    W_g2 = np.asarray(inputs["W_g2"], np.float32); b_g2 = np.asarray(inputs["b_g2"], np.float32)
    W_kg = np.asarray(inputs["W_kg"], np.float32)
    W_vg = np.asarray(inputs["W_vg"], np.float32)
    W_q = np.asarray(inputs["W_q"], np.float32)
    W_k = np.asarray(inputs["W_k"], np.float32)
    W_v = np.asarray(inputs["W_v"], np.float32)

    # per-batch global-slot constants
    q_attn = lat_rep @ W_q.T                      # [B, DIM]
    k_g = lat_rep @ W_kg.T
    v_g = lat_rep @ W_vg.T
    tg = q_attn - k_g
    g1g = np.maximum(tg @ W_g1.T + b_g1, 0.0)
    logit_g = g1g @ W_g2.T + b_g2
    exp_g = np.exp(logit_g)                       # [B, DIM]
    egv = exp_g * v_g
    # fold q_attn into the g1 bias: g1 = relu(W_g1 @ (pos - k) + b_g1')
    b_g1p = b_g1[None, :] + q_attn @ W_g1.T       # [B, DIM]

    # constants
    qp = np.arange(128)
    esel = (qp[:, None] % 16 == qp[None, :] % 16).astype(np.float16)  # [q',p]
    masks = np.zeros((2, 128, 256), np.float16)
    g_of = qp // 16                               # q' // 16 in 0..7
    for t in range(2):
        for nb in range(16):
            for g in range(16):
                masks[t, :, nb * 16 + g] = (g_of == (g - t * 8)).astype(np.float16)
    ident16 = np.eye(128, dtype=np.float16)
    ident32 = np.eye(128, dtype=np.float32)
    iota = np.tile(np.tile(np.arange(CHUNK, dtype=np.int32), 2), (128, 1))

    wd1_l = np.concatenate([W_d1.T, b_d1[None, :]], axis=0).astype(np.float16)  # [4,128]

    maps = []
    for core in range(8):
        b, h = core // 2, core % 2
        qsl = slice(h * NQC, (h + 1) * NQC)
        xq = xyz_q[b, qsl]                        # [2048, 3]
        xn = xyz[b]                               # [2048, 3]
        qx4 = np.concatenate([2.0 * xq.T, np.ones((1, NQC), np.float32)], axis=0)
        xt4 = np.concatenate([xn.T, -np.sum(xn * xn, axis=1)[None, :]], axis=0)
        xyzq4 = np.concatenate([xq.T, np.ones((1, NQC), np.float32)], axis=0).astype(np.float16)
        xyzn4 = np.concatenate([xn.T, np.zeros((1, N), np.float32)], axis=0).astype(np.float16)
        pT = points[b].T.astype(np.float16)          # [256, N]
        ptsT = np.concatenate([pT[0:128], pT[128:256]], axis=1)  # [128, 2N]
        colv = np.stack([b_d2, b_g1p[b], b_g2, exp_g[b], egv[b]],
                        axis=1).astype(np.float32)
        maps.append({
            "qx4": np.ascontiguousarray(qx4, np.float32),
            "xt4": np.ascontiguousarray(xt4, np.float32),
            "ptsT": np.ascontiguousarray(ptsT),
            "xyzq4": np.ascontiguousarray(xyzq4),
            "xyzn4": np.ascontiguousarray(xyzn4),
            "wk_l": np.ascontiguousarray(np.concatenate(
                [W_k.T[0:128], W_k.T[128:256]], axis=1).astype(np.float16)),
            "wv_l": np.ascontiguousarray(np.concatenate(
                [W_v.T[0:128], W_v.T[128:256]], axis=1).astype(np.float16)),
            "wd1_l": np.ascontiguousarray(wd1_l),
            "wd2_l": np.ascontiguousarray(W_d2.T.astype(np.float16)),
            "wg1_l": np.ascontiguousarray(W_g1.T.astype(np.float16)),
            "wg2_l": np.ascontiguousarray(W_g2.T.astype(np.float16)),
            "colv": np.ascontiguousarray(colv),
            "esel": np.ascontiguousarray(esel),
            "masks": np.ascontiguousarray(
                np.concatenate([masks[0], masks[1]], axis=1)),
            "ident16": ident16,
            "ident32": ident32,
            "iota_d": iota,
        })
    return maps


def kernel(**inputs):
    if "nc" not in _CACHE:
        _CACHE["nc"] = _build()
    nc = _CACHE["nc"]
    maps = _host_prep(inputs)
    res = run_bass_kernel_spmd(nc, maps, core_ids=list(range(8)))
    out = np.empty((B, NQ, DIM), np.float32)
    for core in range(8):
        b, h = core // 2, core % 2
        out[b, h * NQC:(h + 1) * NQC, :] = res.results[core]["out"]
    return out
